# revision 1
# baseline (speedup 1.0000x reference)
"""Trainium2 Bass kernel for nn_LogicLayer (differentiable logic-gate layer).

Reference computation:
    a = x[:, idx_a]; b = x[:, idx_b]                  # [B, OUT] gathers
    w = softmax(weights, -1)                          # [OUT, 16]
    out = sum_k w[:, k] * gate_k(a, b)

Every gate value is of the form c0 + c1*a + c2*b + c3*a*b, so
    out[i, j] = W0[j] + W1[j]*a + W2[j]*b + W3[j]*a*b
with W = softmax(weights) @ C, C the [16, 4] gate-coefficient table.

Kernel strategy (data-parallel over batch across 8 cores, 256 rows/core):
  1. softmax+C projection on device -> W0..W3 tiles kept in SBUF
     (heavy reduces on GPSIMD so they overlap the x loads on DMA)
  2. PE-transpose the core's x shard [256, 8192] -> xT [8192, 256] in DRAM
     (stores batched 4 row-blocks per DMA to keep HWDGE off the critical path)
  3. dma_gather rows of xT for idx_a / idx_b (out_dim lands on partitions)
  4. u = W3*a + W2 (ACT), v = W1*a + W0 (DVE ts), t = u*b (DVE/Pool tt)
  5. out = t + v realized directly in PSUM by PE transpose-accumulate
     (two is_transpose matmuls into the same bank), copied back and stored
     in natural [256, 8192] layout.
"""

import numpy as np

# ---------------------------------------------------------------- constants
B_TOT, IN_DIM, OUT_DIM = 2048, 8192, 8192
NCORES = 8

# value = c0 + c1*a + c2*b + c3*ab  for each of the 16 gates
GATE_C = np.array(
    [
        # c0  c1  c2  c3
        [0, 0, 0, 0],    # 0  False
        [0, 0, 0, 1],    # 1  a AND b
        [0, 1, 0, -1],   # 2  a AND NOT b
        [0, 1, 0, 0],    # 3  a
        [0, 0, 1, -1],   # 4  NOT a AND b
        [0, 0, 1, 0],    # 5  b
        [0, 1, 1, -2],   # 6  a XOR b
        [0, 1, 1, -1],   # 7  a OR b
        [1, -1, -1, 1],  # 8  NOT (a OR b)
        [1, -1, -1, 2],  # 9  NOT (a XOR b)
        [1, 0, -1, 0],   # 10 NOT b
        [1, 0, -1, 1],   # 11 a OR NOT b
        [1, -1, 0, 0],   # 12 NOT a
        [1, -1, 0, 1],   # 13 NOT a OR b
        [1, 0, 0, -1],   # 14 NOT (a AND b)
        [1, 0, 0, 0],    # 15 True
    ],
    dtype=np.float32,
)  # [16, 4]


# ---------------------------------------------------------------- device IR
def build_nc(B=B_TOT // NCORES, IN=IN_DIM, OUT=OUT_DIM, NJ=1024):
    """Build the per-core Bass module (SPMD; all cores run the same IR)."""
    import sys

    if "/opt/trn_rl_repo" not in sys.path:
        sys.path.insert(0, "/opt/trn_rl_repo")

    import concourse.tile as tile
    from concourse import bacc, mybir
    from concourse.masks import make_identity
    from contextlib import ExitStack

    f32 = mybir.dt.float32
    i16 = mybir.dt.int16
    PB = B // 128          # batch partition-blocks
    NCH = OUT // NJ        # out_dim chunks
    SLOTS = NJ // 128      # 128-wide j-slots per chunk
    RPT = OUT // 128       # = NCH * SLOTS  (W free dim per partition)
    LCH = min(IN, 2048)    # x load chunk (columns)
    SG = 4                 # xT row-blocks batched per store

    nc = bacc.Bacc("TRN2", target_bir_lowering=False)
    x = nc.declare_dram_parameter("x", [B, IN], f32, isOutput=False)
    wgt = nc.declare_dram_parameter("wgt_shuf", [128, RPT * 16], f32, isOutput=False)
    cg = nc.declare_dram_parameter("cgate", [128, 64], f32, isOutput=False)
    idxa = nc.declare_dram_parameter("idxa16", [128, OUT // 16], i16, isOutput=False)
    idxb = nc.declare_dram_parameter("idxb16", [128, OUT // 16], i16, isOutput=False)
    out = nc.declare_dram_parameter("out", [B, OUT], f32, isOutput=True)

    Ident = mybir.ActivationFunctionType.Identity
    Exp = mybir.ActivationFunctionType.Exp
    MULT = mybir.AluOpType.mult
    ADD = mybir.AluOpType.add

    with tile.TileContext(nc) as tc, ExitStack() as ctx:
        dram = ctx.enter_context(tc.tile_pool(name="dram", bufs=1, space="DRAM"))
        xT = dram.tile([IN, B], f32, name="xT")

        cpool = ctx.enter_context(tc.tile_pool(name="consts", bufs=1))
        xs_stack = ExitStack()
        xs_pool = xs_stack.enter_context(tc.tile_pool(name="xs", bufs=1, side="right"))
        idx_pool = ctx.enter_context(tc.tile_pool(name="idxp", bufs=1))

        # wgt first (small) so the W-phase chain starts immediately,
        # then x shard loads saturate DMA while W-phase compute runs
        cgt = cpool.tile([128, 64], f32, name="cgt")
        nc.sync.dma_start(cgt[:], cg[:])
        wpool = ctx.enter_context(tc.tile_pool(name="wtmp", bufs=2))
        wtile = wpool.tile([128, RPT * 16], f32, name="wtile")
        nc.sync.dma_start(wtile[:], wgt[:])
        xh = {}
        for c0 in range(IN // LCH):
            for h in range(PB):
                xht = xs_pool.tile([128, LCH], f32, name=f"xh{h}_{c0}",
                                   tag=f"xh{h}_{c0}")
                nc.sync.dma_start(xht[:], x[h * 128:(h + 1) * 128,
                                            c0 * LCH:(c0 + 1) * LCH])
                xh[h, c0] = xht
        idxa_sb = idx_pool.tile([128, OUT // 16], i16, name="idxa_sb")
        nc.sync.dma_start(idxa_sb[:], idxa[:])
        idxb_sb = idx_pool.tile([128, OUT // 16], i16, name="idxb_sb")
        nc.sync.dma_start(idxb_sb[:], idxb[:])

        ident = cpool.tile([128, 128], f32, name="ident")
        make_identity(nc, ident[:])

        # ---- W = softmax(weights) @ C, in (q, r) layout: j = r*128 + q ----
        # heavy elementwise on GPSIMD so DVE stays free for phase-A copybacks
        wk = [cpool.tile([128, RPT], f32, name=f"wk{k}") for k in range(4)]
        if True:
            wexp = wpool.tile([128, RPT * 16], f32, name="wexp")
            nc.scalar.activation(wexp[:], wtile[:], Exp)
            wsum = wpool.tile([128, RPT], f32, name="wsum")
            nc.vector.tensor_reduce(
                out=wsum[:],
                in_=wexp[:].rearrange("p (r k) -> p r k", k=16),
                op=ADD,
                axis=mybir.AxisListType.X,
            )
            wrcp = wpool.tile([128, RPT], f32, name="wrcp")
            nc.vector.reciprocal(wrcp[:], wsum[:])
            for k in range(4):
                wtmp = wpool.tile([128, RPT * 16], f32, name="wtmp", tag="wtmp")
                ck_bcast = (
                    cgt[:, k * 16:(k + 1) * 16]
                    .rearrange("p (r k) -> p r k", r=1)
                    .to_broadcast([128, RPT, 16])
                )
                nc.gpsimd.tensor_tensor(
                    out=wtmp[:].rearrange("p (r k) -> p r k", k=16),
                    in0=wexp[:].rearrange("p (r k) -> p r k", k=16),
                    in1=ck_bcast,
                    op=MULT,
                )
                wred = wpool.tile([128, RPT], f32, name="wred", tag="wred")
                nc.vector.tensor_reduce(
                    out=wred[:],
                    in_=wtmp[:].rearrange("p (r k) -> p r k", k=16),
                    op=ADD,
                    axis=mybir.AxisListType.X,
                )
                nc.vector.tensor_tensor(out=wk[k][:], in0=wred[:], in1=wrcp[:],
                                        op=MULT)

        # ---- phase A: transpose x shard into xT (DRAM) ----
        psumT = ctx.enter_context(tc.tile_pool(name="psumT", bufs=4, space="PSUM"))
        stg_pool = ctx.enter_context(tc.tile_pool(name="xstg", bufs=3))
        if True:
            for g in range(IN // (SG * 128)):
                st = stg_pool.tile([128, SG, B], f32, tag="st")
                for i in range(SG):
                    cb = g * SG + i
                    c0, cc = (cb * 128) // LCH, (cb * 128) % LCH
                    pt = psumT.tile([128, B], f32, tag="pt")
                    for h in range(PB):
                        nc.tensor.transpose(
                            pt[:, h * 128:(h + 1) * 128],
                            xh[h, c0][:, cc:cc + 128],
                            ident[:],
                        )
                    if cb % 2 == 0:
                        nc.vector.tensor_copy(st[:, i, :], pt[:])
                    else:
                        nc.scalar.copy(st[:, i, :], pt[:])
                nc.sync.dma_start(
                    xT[g * SG * 128:(g + 1) * SG * 128, :]
                    .rearrange("(i p) b -> p i b", p=128),
                    st[:],
                )

        xs_stack.close()  # release x tiles; phase-B pools reuse the zone

        # ---- phase B: gather + gates + transpose-back ----
        gpool = ctx.enter_context(tc.tile_pool(name="gath", bufs=4))
        uvpool = ctx.enter_context(tc.tile_pool(name="uv", bufs=12))
        psumO = ctx.enter_context(tc.tile_pool(name="psumO", bufs=4, space="PSUM"))
        ostg = ctx.enter_context(tc.tile_pool(name="ostg", bufs=3))
        if True:
            NJ16 = NJ // 16
            for ck in range(NCH):
                ga = gpool.tile([128, SLOTS, B], f32, tag="ga")
                nc.gpsimd.dma_gather(
                    ga[:], xT[:], idxa_sb[:, ck * NJ16:(ck + 1) * NJ16], NJ, NJ, B
                )
                gb = gpool.tile([128, SLOTS, B], f32, tag="gb")
                nc.gpsimd.dma_gather(
                    gb[:], xT[:], idxb_sb[:, ck * NJ16:(ck + 1) * NJ16], NJ, NJ, B
                )
                for cq in range(SLOTS // 4):
                    ts_v, ts_t = [], []
                    for ci in range(4):
                        c = cq * 4 + ci
                        r = ck * SLOTS + c
                        u = uvpool.tile([128, B], f32, tag="u")
                        nc.scalar.activation(
                            u[:], ga[:, c, :], Ident,
                            scale=wk[3][:, r:r + 1], bias=wk[2][:, r:r + 1],
                        )
                        v = uvpool.tile([128, B], f32, tag="v")
                        nc.vector.tensor_scalar(
                            v[:], ga[:, c, :],
                            wk[1][:, r:r + 1], wk[0][:, r:r + 1],
                            op0=MULT, op1=ADD,
                        )
                        t = uvpool.tile([128, B], f32, tag="t")
                        eng = nc.gpsimd if ci == 3 else nc.vector
                        eng.tensor_tensor(t[:], u[:], gb[:, c, :], op=MULT)
                        ts_v.append(v)
                        ts_t.append(t)
                    for h in range(PB):
                        po = psumO.tile([128, 512], f32, tag="po")
                        for ci in range(4):
                            hs = slice(h * 128, (h + 1) * 128)
                            nc.tensor.matmul(
                                po[:, ci * 128:(ci + 1) * 128],
                                ts_t[ci][:, hs], ident[:],
                                is_transpose=True, start=True, stop=False,
                            )
                            nc.tensor.matmul(
                                po[:, ci * 128:(ci + 1) * 128],
                                ts_v[ci][:, hs], ident[:],
                                is_transpose=True, start=False, stop=True,
                            )
                        og = ostg.tile([128, 512], f32, tag="og")
                        if (h + cq) % 2 == 0:
                            nc.vector.tensor_copy(og[:], po[:])
                        else:
                            nc.scalar.copy(og[:], po[:])
                        j0 = ck * NJ + cq * 512
                        nc.sync.dma_start(
                            out[h * 128:(h + 1) * 128, j0:j0 + 512], og[:]
                        )
    nc.compile()
    return nc


# ---------------------------------------------------------------- host side
def _wrap_idx(idx, OUT, NJ):
    """Pack an index vector into dma_gather's wrapped int16 layout.

    Per chunk ck the NJ indices live in columns [ck*NJ/16, (ck+1)*NJ/16):
    idx16[p, ck*NJ/16 + s] = idx[ck*NJ + s*16 + p%16], replicated over the
    8 groups of 16 partitions.
    """
    nch = OUT // NJ
    a = np.asarray(idx).astype(np.int16).reshape(nch, NJ // 16, 16)  # [ck, s, p]
    a = a.transpose(2, 0, 1).reshape(16, nch * (NJ // 16))           # [p, ck*s]
    return np.ascontiguousarray(np.tile(a, (8, 1)))                  # [128, ...]


def _prep_inputs(x, weights, idx_a, idx_b, NJ=1024):
    B = B_TOT // NCORES
    NCH = OUT_DIM // NJ
    SLOTS = NJ // 128
    x = np.asarray(x, dtype=np.float32)
    weights = np.asarray(weights, dtype=np.float32)
    # wgt_shuf[q, (ck*SLOTS+c)*16+k] = weights[ck*NJ + c*128 + q, k]
    wgt_shuf = np.ascontiguousarray(
        weights.reshape(NCH, SLOTS, 128, 16).transpose(2, 0, 1, 3).reshape(128, -1)
    )
    cgate = np.ascontiguousarray(np.tile(GATE_C.T.reshape(1, 64), (128, 1)))
    ia = _wrap_idx(idx_a, OUT_DIM, NJ)
    ib = _wrap_idx(idx_b, OUT_DIM, NJ)
    in_maps = []
    for c in range(NCORES):
        in_maps.append(
            {
                "x": np.ascontiguousarray(x[c * B:(c + 1) * B]),
                "wgt_shuf": wgt_shuf,
                "cgate": cgate,
                "idxa16": ia,
                "idxb16": ib,
            }
        )
    return in_maps


_NC_CACHE = {}


def _get_nc():
    if "nc" not in _NC_CACHE:
        _NC_CACHE["nc"] = build_nc()
    return _NC_CACHE["nc"]


def kernel(x, weights, idx_a, idx_b):
    import sys

    if "/opt/trn_rl_repo" not in sys.path:
        sys.path.insert(0, "/opt/trn_rl_repo")
    from concourse.bass_utils import run_bass_kernel_spmd

    nc = _get_nc()
    in_maps = _prep_inputs(x, weights, idx_a, idx_b)
    res = run_bass_kernel_spmd(nc, in_maps, list(range(NCORES)))
    return np.concatenate([r["out"] for r in res.results], axis=0)


if __name__ == "__main__":
    nc = build_nc()
    print("built OK")



# revision 2
# speedup vs baseline: 2.4758x; 2.4758x over previous
"""Trainium2 Bass kernel for nn_LogicLayer (differentiable logic-gate layer).

Reference computation:
    a = x[:, idx_a]; b = x[:, idx_b]                  # [B, OUT] gathers
    w = softmax(weights, -1)                          # [OUT, 16]
    out = sum_k w[:, k] * gate_k(a, b)

Every gate value is of the form c0 + c1*a + c2*b + c3*a*b, so
    out[i, j] = W0[j] + W1[j]*a + W2[j]*b + W3[j]*a*b
with W = softmax(weights) @ C, C the [16, 4] gate-coefficient table.

Kernel strategy (data-parallel over batch across 8 cores, 256 rows/core):
  - host prep: per-core x shard is laid out feature-major as bf16
    (xT[j, i] = x[i, j]) so each neuron's input column is one contiguous
    512 B row; indices are packed into dma_gather's wrapped int16 layout
    with idx_a/idx_b of each 512-neuron group combined into one gather.
  - device: W = softmax(weights) @ C on ACT/DVE (overlaps the first
    gathers), then per 512-neuron group one SWDGE dma_gather pulls the
    a- and b-columns (bf16, neurons on partitions), ACT computes
    u = W3*a + W2, DVE computes v = W1*a + W0, t = u*b, pre = t + v,
    and PE transposes pre back to batch-major via identity matmuls into
    PSUM; ACT/DVE copy the f32 result to SBUF and HWDGE stores it.
HBM traffic per core: 8 MiB gather reads + 8 MiB output stores.
"""

import numpy as np

# ---------------------------------------------------------------- constants
B_TOT, IN_DIM, OUT_DIM = 2048, 8192, 8192
NCORES = 8
B = B_TOT // NCORES          # 256 batch rows per core
NG = 16                      # gather groups (512 neurons each)
GJ = OUT_DIM // NG           # 512 neurons per gather
RPT = OUT_DIM // 128         # 64 (W free dim per partition)

# value = c0 + c1*a + c2*b + c3*ab  for each of the 16 gates
GATE_C = np.array(
    [
        # c0  c1  c2  c3
        [0, 0, 0, 0],    # 0  False
        [0, 0, 0, 1],    # 1  a AND b
        [0, 1, 0, -1],   # 2  a AND NOT b
        [0, 1, 0, 0],    # 3  a
        [0, 0, 1, -1],   # 4  NOT a AND b
        [0, 0, 1, 0],    # 5  b
        [0, 1, 1, -2],   # 6  a XOR b
        [0, 1, 1, -1],   # 7  a OR b
        [1, -1, -1, 1],  # 8  NOT (a OR b)
        [1, -1, -1, 2],  # 9  NOT (a XOR b)
        [1, 0, -1, 0],   # 10 NOT b
        [1, 0, -1, 1],   # 11 a OR NOT b
        [1, -1, 0, 0],   # 12 NOT a
        [1, -1, 0, 1],   # 13 NOT a OR b
        [1, 0, 0, -1],   # 14 NOT (a AND b)
        [1, 0, 0, 0],    # 15 True
    ],
    dtype=np.float32,
)  # [16, 4]


# ---------------------------------------------------------------- device IR
def build_nc():
    """Build the per-core Bass module (SPMD; all cores run the same IR)."""
    import sys

    if "/opt/trn_rl_repo" not in sys.path:
        sys.path.insert(0, "/opt/trn_rl_repo")

    import concourse.tile as tile
    from concourse import bacc, mybir
    from concourse.masks import make_identity
    from contextlib import ExitStack

    f32 = mybir.dt.float32
    bf16 = mybir.dt.bfloat16
    i16 = mybir.dt.int16

    nc = bacc.Bacc("TRN2", target_bir_lowering=False)
    xT = nc.declare_dram_parameter("xTb", [IN_DIM, B], bf16, isOutput=False)
    wgt = nc.declare_dram_parameter("wgt_shuf", [128, RPT * 16], f32, isOutput=False)
    cg = nc.declare_dram_parameter("cgate", [128, 64], f32, isOutput=False)
    idxab = nc.declare_dram_parameter("idxab16", [128, NG * 64], i16, isOutput=False)
    out = nc.declare_dram_parameter("out", [B, OUT_DIM], f32, isOutput=True)

    Ident = mybir.ActivationFunctionType.Identity
    Exp = mybir.ActivationFunctionType.Exp
    MULT = mybir.AluOpType.mult
    ADD = mybir.AluOpType.add

    with tile.TileContext(nc) as tc, ExitStack() as ctx:
        cpool = ctx.enter_context(tc.tile_pool(name="consts", bufs=1))
        idx_pool = ctx.enter_context(tc.tile_pool(name="idxp", bufs=1))
        wpool = ctx.enter_context(tc.tile_pool(name="wtmp", bufs=2))

        # small loads first so the W-phase chain starts immediately
        cgt = cpool.tile([128, 64], f32, name="cgt")
        nc.sync.dma_start(cgt[:], cg[:])
        wtile = wpool.tile([128, RPT * 16], f32, name="wtile")
        nc.sync.dma_start(wtile[:], wgt[:])
        idx_sb = idx_pool.tile([128, NG * 64], i16, name="idx_sb")
        nc.sync.dma_start(idx_sb[:], idxab[:])

        identb = cpool.tile([128, 128], bf16, name="identb")
        make_identity(nc, identb[:])

        # ---- gathers: one per 512-neuron group, a+b combined ----------
        # issued as early as possible; Q7 blocks in await_space while the
        # 16-deep-per-engine ring drains, which is fine (nothing else runs
        # on GpSimd). Consumers wake per-gather via tile dependencies.
        gpool = ctx.enter_context(tc.tile_pool(name="gath", bufs=4))
        gt = []
        for gk in range(NG):
            g = gpool.tile([128, 8, B], bf16, name=f"g{gk}", tag="g")
            nc.gpsimd.dma_gather(
                g[:], xT[:], idx_sb[:, gk * 64:(gk + 1) * 64], 2 * GJ, 2 * GJ, B
            )
            gt.append(g)

        # ---- W = softmax(weights) @ C, in (q, r) layout: j = r*128 + q ----
        wk = [cpool.tile([128, RPT], f32, name=f"wk{k}") for k in range(4)]
        wexp = wpool.tile([128, RPT * 16], f32, name="wexp")
        nc.scalar.activation(wexp[:], wtile[:], Exp)
        wsum = wpool.tile([128, RPT], f32, name="wsum")
        nc.vector.tensor_reduce(
            out=wsum[:],
            in_=wexp[:].rearrange("p (r k) -> p r k", k=16),
            op=ADD,
            axis=mybir.AxisListType.X,
        )
        wrcp = wpool.tile([128, RPT], f32, name="wrcp")
        nc.vector.reciprocal(wrcp[:], wsum[:])
        for k in range(4):
            wtmp = wpool.tile([128, RPT * 16], f32, name="wtmp", tag="wtmp")
            ck_bcast = (
                cgt[:, k * 16:(k + 1) * 16]
                .rearrange("p (r k) -> p r k", r=1)
                .to_broadcast([128, RPT, 16])
            )
            nc.vector.tensor_tensor(
                out=wtmp[:].rearrange("p (r k) -> p r k", k=16),
                in0=wexp[:].rearrange("p (r k) -> p r k", k=16),
                in1=ck_bcast,
                op=MULT,
            )
            wred = wpool.tile([128, RPT], f32, name="wred", tag="wred")
            nc.vector.tensor_reduce(
                out=wred[:],
                in_=wtmp[:].rearrange("p (r k) -> p r k", k=16),
                op=ADD,
                axis=mybir.AxisListType.X,
            )
            nc.vector.tensor_tensor(out=wk[k][:], in0=wred[:], in1=wrcp[:],
                                    op=MULT)

        # ---- per-group gates + transpose-back -------------------------
        uvpool = ctx.enter_context(tc.tile_pool(name="uv", bufs=8))
        prepool = ctx.enter_context(tc.tile_pool(name="pre", bufs=20))
        psumO = ctx.enter_context(tc.tile_pool(name="psumO", bufs=4, space="PSUM"))
        ostg = ctx.enter_context(tc.tile_pool(name="ostg", bufs=4))

        pre = [None] * 8
        for gk in range(NG):
            g = gt[gk]
            for ci in range(4):
                r = gk * 4 + ci
                u = uvpool.tile([128, B], bf16, tag="u")
                nc.scalar.activation(
                    u[:], g[:, ci, :], Ident,
                    scale=wk[3][:, r:r + 1], bias=wk[2][:, r:r + 1],
                )
                v = uvpool.tile([128, B], bf16, tag="v")
                nc.vector.tensor_scalar(
                    v[:], g[:, ci, :],
                    wk[1][:, r:r + 1], wk[0][:, r:r + 1],
                    op0=MULT, op1=ADD,
                )
                t = uvpool.tile([128, B], bf16, tag="t")
                nc.vector.tensor_tensor(t[:], u[:], g[:, 4 + ci, :], op=MULT)
                p = prepool.tile([128, B], bf16, tag="p")
                nc.vector.tensor_tensor(p[:], t[:], v[:], op=ADD)
                pre[(gk % 2) * 4 + ci] = p
            if gk % 2 == 1:
                ck = gk // 2
                for h in range(B // 128):
                    po = psumO.tile([128, 1024], bf16, tag="po")
                    for s in range(8):
                        nc.tensor.transpose(
                            po[:, s * 128:(s + 1) * 128],
                            pre[s][:, h * 128:(h + 1) * 128],
                            identb[:],
                        )
                    og = ostg.tile([128, 1024], f32, tag="og")
                    if h == 0:
                        nc.scalar.copy(og[:], po[:])
                    else:
                        nc.vector.tensor_copy(og[:], po[:])
                    nc.sync.dma_start(
                        out[h * 128:(h + 1) * 128,
                            ck * 1024:(ck + 1) * 1024], og[:]
                    )
    nc.compile()
    return nc


# ---------------------------------------------------------------- host side
def _wrap_idx_groups(idx_a, idx_b):
    """Pack combined a/b indices into dma_gather's wrapped int16 layout.

    Gather gk covers neurons [gk*GJ, (gk+1)*GJ) and pulls 2*GJ rows: the GJ
    idx_a rows then the GJ idx_b rows. Within a gather, unwrapped position
    i = s*16 + p reads idx16[p % 16, gk*64 + s]; replicated over the 8
    groups of 16 partitions.
    """
    a = np.asarray(idx_a).astype(np.int16)
    b = np.asarray(idx_b).astype(np.int16)
    comb = np.stack(
        [np.concatenate([a[gk * GJ:(gk + 1) * GJ], b[gk * GJ:(gk + 1) * GJ]])
         for gk in range(NG)]
    )  # [NG, 2*GJ]
    w = comb.reshape(NG, 64, 16).transpose(2, 0, 1).reshape(16, NG * 64)
    return np.ascontiguousarray(np.tile(w, (8, 1)))  # [128, NG*64]


def _prep_inputs(x, weights, idx_a, idx_b):
    import ml_dtypes

    x = np.asarray(x, dtype=np.float32)
    weights = np.asarray(weights, dtype=np.float32)
    # wgt_shuf[q, r*16+k] = weights[r*128 + q, k]
    wgt_shuf = np.ascontiguousarray(
        weights.reshape(RPT, 128, 16).transpose(1, 0, 2).reshape(128, -1)
    )
    cgate = np.ascontiguousarray(np.tile(GATE_C.T.reshape(1, 64), (128, 1)))
    iw = _wrap_idx_groups(idx_a, idx_b)
    in_maps = []
    for c in range(NCORES):
        xs = x[c * B:(c + 1) * B]                      # [B, IN]
        xTb = np.ascontiguousarray(xs.T.astype(ml_dtypes.bfloat16))
        in_maps.append(
            {
                "xTb": xTb,
                "wgt_shuf": wgt_shuf,
                "cgate": cgate,
                "idxab16": iw,
            }
        )
    return in_maps


_NC_CACHE = {}


def _get_nc():
    if "nc" not in _NC_CACHE:
        _NC_CACHE["nc"] = build_nc()
    return _NC_CACHE["nc"]


def kernel(x, weights, idx_a, idx_b):
    import sys

    if "/opt/trn_rl_repo" not in sys.path:
        sys.path.insert(0, "/opt/trn_rl_repo")
    from concourse.bass_utils import run_bass_kernel_spmd

    nc = _get_nc()
    in_maps = _prep_inputs(x, weights, idx_a, idx_b)
    res = run_bass_kernel_spmd(nc, in_maps, list(range(NCORES)))
    return np.concatenate([r["out"] for r in res.results], axis=0)


if __name__ == "__main__":
    nc = build_nc()
    print("built OK")


# revision 3
# speedup vs baseline: 5.5460x; 2.2400x over previous
"""Trainium2 Bass kernel for nn_LogicLayer (differentiable logic-gate layer).

Reference computation:
    a = x[:, idx_a]; b = x[:, idx_b]                  # [B, OUT] gathers
    w = softmax(weights, -1)                          # [OUT, 16]
    out = sum_k w[:, k] * gate_k(a, b)

Every gate value is of the form c0 + c1*a + c2*b + c3*a*b, so
    out[i, j] = W0[j] + W1[j]*a + W2[j]*b + W3[j]*a*b
with W = softmax(weights) @ C, C the [16, 4] gate-coefficient table.

Sharding: out_dim-parallel across 8 cores (1024 neurons each, full 2048
batch), x feature-major (transposed) in bf16 and replicated. This keeps
the SWDGE gather descriptor count per core at 2048 (the Q7 descriptor
loop costs ~8.5 ns/index, so batch-parallel's 16K indices/core was the
bottleneck) while HBM traffic stays 8 MiB gathers + 8 MiB stores/core.

Device pipeline per core:
  - W = softmax(weights) @ C on ACT/DVE (overlaps the first gathers)
  - 4 SWDGE dma_gathers, each pulling 256 neurons' a- and b-columns
    (512 indices x 4 KiB bf16 rows, neurons land on partitions)
  - per 128-neuron slot: u = W3*a + W2 (ACT), v = W1*a + W0 (DVE),
    t = u*b (DVE), pre = t + v (DVE), all bf16
  - PE transposes pre back to batch-major (identity matmuls, bf16 PSUM),
    ACT/DVE copy-cast to f32, HWDGE stores [128, 512] blocks.
"""

import numpy as np

# ---------------------------------------------------------------- constants
B_TOT, IN_DIM, OUT_DIM = 2048, 8192, 8192
NCORES = 8
OC = OUT_DIM // NCORES       # 1024 neurons per core
NG = 4                       # gather groups per core (256 neurons each)
GJ = OC // NG                # 256 neurons per gather
RPT = OC // 128              # 8 (W free dim per partition)
NH = B_TOT // 128            # 16 batch blocks

# value = c0 + c1*a + c2*b + c3*ab  for each of the 16 gates
GATE_C = np.array(
    [
        # c0  c1  c2  c3
        [0, 0, 0, 0],    # 0  False
        [0, 0, 0, 1],    # 1  a AND b
        [0, 1, 0, -1],   # 2  a AND NOT b
        [0, 1, 0, 0],    # 3  a
        [0, 0, 1, -1],   # 4  NOT a AND b
        [0, 0, 1, 0],    # 5  b
        [0, 1, 1, -2],   # 6  a XOR b
        [0, 1, 1, -1],   # 7  a OR b
        [1, -1, -1, 1],  # 8  NOT (a OR b)
        [1, -1, -1, 2],  # 9  NOT (a XOR b)
        [1, 0, -1, 0],   # 10 NOT b
        [1, 0, -1, 1],   # 11 a OR NOT b
        [1, -1, 0, 0],   # 12 NOT a
        [1, -1, 0, 1],   # 13 NOT a OR b
        [1, 0, 0, -1],   # 14 NOT (a AND b)
        [1, 0, 0, 0],    # 15 True
    ],
    dtype=np.float32,
)  # [16, 4]


# ---------------------------------------------------------------- device IR
def build_nc():
    """Build the per-core Bass module (SPMD; all cores run the same IR)."""
    import sys

    if "/opt/trn_rl_repo" not in sys.path:
        sys.path.insert(0, "/opt/trn_rl_repo")

    import concourse.tile as tile
    from concourse import bacc, mybir
    from concourse.masks import make_identity
    from contextlib import ExitStack

    f32 = mybir.dt.float32
    bf16 = mybir.dt.bfloat16
    i16 = mybir.dt.int16
    B = B_TOT

    nc = bacc.Bacc("TRN2", target_bir_lowering=False)
    xT = nc.declare_dram_parameter("xTb", [IN_DIM, B], bf16, isOutput=False)
    wgt = nc.declare_dram_parameter("wgt_shuf", [128, RPT * 16], f32, isOutput=False)
    cg = nc.declare_dram_parameter("cgate", [128, 64], f32, isOutput=False)
    idxab = nc.declare_dram_parameter("idxab16", [128, NG * 32], i16, isOutput=False)
    out = nc.declare_dram_parameter("out", [B, OC], f32, isOutput=True)

    Ident = mybir.ActivationFunctionType.Identity
    Exp = mybir.ActivationFunctionType.Exp
    MULT = mybir.AluOpType.mult
    ADD = mybir.AluOpType.add

    with tile.TileContext(nc) as tc, ExitStack() as ctx:
        cpool = ctx.enter_context(tc.tile_pool(name="consts", bufs=1))
        idx_pool = ctx.enter_context(tc.tile_pool(name="idxp", bufs=1))
        wpool = ctx.enter_context(tc.tile_pool(name="wtmp", bufs=2))

        # small loads first so the W-phase chain starts immediately
        cgt = cpool.tile([128, 64], f32, name="cgt")
        nc.sync.dma_start(cgt[:], cg[:])
        wtile = wpool.tile([128, RPT * 16], f32, name="wtile")
        nc.sync.dma_start(wtile[:], wgt[:])
        idx_sb = idx_pool.tile([128, NG * 32], i16, name="idx_sb")
        nc.sync.dma_start(idx_sb[:], idxab[:])

        identb = cpool.tile([128, 128], bf16, name="identb")
        make_identity(nc, identb[:])

        # ---- gathers: one per 256-neuron group, a+b combined ----------
        gpool = ctx.enter_context(tc.tile_pool(name="gath", bufs=4))
        gt = []
        for gk in range(NG):
            g = gpool.tile([128, 4, B], bf16, name=f"g{gk}", tag="g")
            nc.gpsimd.dma_gather(
                g[:], xT[:], idx_sb[:, gk * 32:(gk + 1) * 32], 2 * GJ, 2 * GJ, B
            )
            gt.append(g)

        # ---- W = softmax(weights) @ C, in (q, r) layout: j = r*128 + q ----
        wk = [cpool.tile([128, RPT], f32, name=f"wk{k}") for k in range(4)]
        wexp = wpool.tile([128, RPT * 16], f32, name="wexp")
        nc.scalar.activation(wexp[:], wtile[:], Exp)
        wsum = wpool.tile([128, RPT], f32, name="wsum")
        nc.vector.tensor_reduce(
            out=wsum[:],
            in_=wexp[:].rearrange("p (r k) -> p r k", k=16),
            op=ADD,
            axis=mybir.AxisListType.X,
        )
        wrcp = wpool.tile([128, RPT], f32, name="wrcp")
        nc.vector.reciprocal(wrcp[:], wsum[:])
        for k in range(4):
            wtmp = wpool.tile([128, RPT * 16], f32, name="wtmp", tag="wtmp")
            ck_bcast = (
                cgt[:, k * 16:(k + 1) * 16]
                .rearrange("p (r k) -> p r k", r=1)
                .to_broadcast([128, RPT, 16])
            )
            nc.vector.tensor_tensor(
                out=wtmp[:].rearrange("p (r k) -> p r k", k=16),
                in0=wexp[:].rearrange("p (r k) -> p r k", k=16),
                in1=ck_bcast,
                op=MULT,
            )
            wred = wpool.tile([128, RPT], f32, name="wred", tag="wred")
            nc.vector.tensor_reduce(
                out=wred[:],
                in_=wtmp[:].rearrange("p (r k) -> p r k", k=16),
                op=ADD,
                axis=mybir.AxisListType.X,
            )
            nc.vector.tensor_tensor(out=wk[k][:], in0=wred[:], in1=wrcp[:],
                                    op=MULT)

        # ---- per-slot gates + transpose-back --------------------------
        uvpool = ctx.enter_context(tc.tile_pool(name="uv", bufs=6))
        prepool = ctx.enter_context(tc.tile_pool(name="pre", bufs=10))
        psumO = ctx.enter_context(tc.tile_pool(name="psumO", bufs=4, space="PSUM"))
        ostg = ctx.enter_context(tc.tile_pool(name="ostg", bufs=4))

        pre = [None] * RPT
        for gk in range(NG):
            g = gt[gk]
            for cj in range(2):
                r = gk * 2 + cj
                u = uvpool.tile([128, B], bf16, tag="u")
                nc.scalar.activation(
                    u[:], g[:, cj, :], Ident,
                    scale=wk[3][:, r:r + 1], bias=wk[2][:, r:r + 1],
                )
                v = uvpool.tile([128, B], bf16, tag="v")
                nc.vector.tensor_scalar(
                    v[:], g[:, cj, :],
                    wk[1][:, r:r + 1], wk[0][:, r:r + 1],
                    op0=MULT, op1=ADD,
                )
                t = uvpool.tile([128, B], bf16, tag="t")
                nc.vector.tensor_tensor(t[:], u[:], g[:, 2 + cj, :], op=MULT)
                p = prepool.tile([128, B], bf16, tag="p")
                nc.vector.tensor_tensor(p[:], t[:], v[:], op=ADD)
                pre[r] = p
            if gk % 2 == 1:
                half = gk // 2
                for h in range(NH):
                    po = psumO.tile([128, 512], bf16, tag="po")
                    for s in range(4):
                        nc.tensor.transpose(
                            po[:, s * 128:(s + 1) * 128],
                            pre[half * 4 + s][:, h * 128:(h + 1) * 128],
                            identb[:],
                        )
                    og = ostg.tile([128, 512], f32, tag="og")
                    if h % 2 == 0:
                        nc.scalar.copy(og[:], po[:])
                    else:
                        nc.vector.tensor_copy(og[:], po[:])
                    nc.sync.dma_start(
                        out[h * 128:(h + 1) * 128,
                            half * 512:(half + 1) * 512], og[:]
                    )
    nc.compile()
    return nc


# ---------------------------------------------------------------- host side
def _wrap_idx_core(ia_core, ib_core):
    """Pack one core's combined a/b indices into the wrapped int16 layout.

    Gather gk covers neurons [gk*GJ, (gk+1)*GJ) of this core's shard and
    pulls 2*GJ rows: the GJ idx_a rows then the GJ idx_b rows. Unwrapped
    position i = s*16 + p reads idx16[p % 16, gk*32 + s]; replicated over
    the 8 groups of 16 partitions.
    """
    comb = np.stack(
        [np.concatenate([ia_core[gk * GJ:(gk + 1) * GJ],
                         ib_core[gk * GJ:(gk + 1) * GJ]])
         for gk in range(NG)]
    ).astype(np.int16)  # [NG, 2*GJ]
    w = comb.reshape(NG, 32, 16).transpose(2, 0, 1).reshape(16, NG * 32)
    return np.ascontiguousarray(np.tile(w, (8, 1)))  # [128, NG*32]


def _prep_inputs(x, weights, idx_a, idx_b):
    import ml_dtypes

    x = np.asarray(x, dtype=np.float32)
    weights = np.asarray(weights, dtype=np.float32)
    idx_a = np.asarray(idx_a)
    idx_b = np.asarray(idx_b)
    xTb = np.ascontiguousarray(x.T.astype(ml_dtypes.bfloat16))  # [IN, B] bf16
    cgate = np.ascontiguousarray(np.tile(GATE_C.T.reshape(1, 64), (128, 1)))
    in_maps = []
    for c in range(NCORES):
        wc = weights[c * OC:(c + 1) * OC]              # [OC, 16]
        wgt_shuf = np.ascontiguousarray(
            wc.reshape(RPT, 128, 16).transpose(1, 0, 2).reshape(128, -1)
        )
        iw = _wrap_idx_core(idx_a[c * OC:(c + 1) * OC],
                            idx_b[c * OC:(c + 1) * OC])
        in_maps.append(
            {
                "xTb": xTb,
                "wgt_shuf": wgt_shuf,
                "cgate": cgate,
                "idxab16": iw,
            }
        )
    return in_maps


_NC_CACHE = {}


def _get_nc():
    if "nc" not in _NC_CACHE:
        _NC_CACHE["nc"] = build_nc()
    return _NC_CACHE["nc"]


def kernel(x, weights, idx_a, idx_b):
    import sys

    if "/opt/trn_rl_repo" not in sys.path:
        sys.path.insert(0, "/opt/trn_rl_repo")
    from concourse.bass_utils import run_bass_kernel_spmd

    nc = _get_nc()
    in_maps = _prep_inputs(x, weights, idx_a, idx_b)
    res = run_bass_kernel_spmd(nc, in_maps, list(range(NCORES)))
    return np.concatenate([r["out"] for r in res.results], axis=1)


if __name__ == "__main__":
    nc = build_nc()
    print("built OK")


# revision 9
# speedup vs baseline: 5.6457x; 1.0180x over previous
"""Trainium2 Bass kernel for nn_LogicLayer (differentiable logic-gate layer).

Reference computation:
    a = x[:, idx_a]; b = x[:, idx_b]                  # [B, OUT] gathers
    w = softmax(weights, -1)                          # [OUT, 16]
    out = sum_k w[:, k] * gate_k(a, b)

Every gate value is of the form c0 + c1*a + c2*b + c3*a*b, so
    out[i, j] = W0[j] + W1[j]*a + W2[j]*b + W3[j]*a*b
with W = softmax(weights) @ C, C the [16, 4] gate-coefficient table.

Sharding: out_dim-parallel across 8 cores (1024 neurons each, full 2048
batch), x feature-major (transposed) in bf16 and replicated. This keeps
the SWDGE gather descriptor count per core at 2048 (the Q7 descriptor
loop costs ~8.5 ns/index, so batch-parallel's 16K indices/core was the
bottleneck) while HBM traffic stays 8 MiB gathers + 8 MiB stores/core.

Device pipeline per core:
  - W = softmax(weights) @ C on ACT/DVE (overlaps the first gathers)
  - 4 SWDGE dma_gathers, each pulling 256 neurons' a- and b-columns
    (512 indices x 4 KiB bf16 rows, neurons land on partitions)
  - per 128-neuron slot: u = W3*a + W2 (ACT), v = W1*a + W0 (DVE),
    t = u*b (DVE), pre = t + v (DVE), all bf16
  - PE transposes pre back to batch-major (identity matmuls, bf16 PSUM),
    ACT/DVE copy-cast to f32, HWDGE stores [128, 512] blocks.
"""

import numpy as np

# ---------------------------------------------------------------- constants
B_TOT, IN_DIM, OUT_DIM = 2048, 8192, 8192
NCORES = 8
OC = OUT_DIM // NCORES       # 1024 neurons per core
NG = 4                       # gather groups per core (256 neurons each)
GJ = OC // NG                # 256 neurons per gather
RPT = OC // 128              # 8 (W free dim per partition)
NH = B_TOT // 128            # 16 batch blocks

# value = c0 + c1*a + c2*b + c3*ab  for each of the 16 gates
GATE_C = np.array(
    [
        # c0  c1  c2  c3
        [0, 0, 0, 0],    # 0  False
        [0, 0, 0, 1],    # 1  a AND b
        [0, 1, 0, -1],   # 2  a AND NOT b
        [0, 1, 0, 0],    # 3  a
        [0, 0, 1, -1],   # 4  NOT a AND b
        [0, 0, 1, 0],    # 5  b
        [0, 1, 1, -2],   # 6  a XOR b
        [0, 1, 1, -1],   # 7  a OR b
        [1, -1, -1, 1],  # 8  NOT (a OR b)
        [1, -1, -1, 2],  # 9  NOT (a XOR b)
        [1, 0, -1, 0],   # 10 NOT b
        [1, 0, -1, 1],   # 11 a OR NOT b
        [1, -1, 0, 0],   # 12 NOT a
        [1, -1, 0, 1],   # 13 NOT a OR b
        [1, 0, 0, -1],   # 14 NOT (a AND b)
        [1, 0, 0, 0],    # 15 True
    ],
    dtype=np.float32,
)  # [16, 4]


# ---------------------------------------------------------------- device IR
def build_nc():
    """Build the per-core Bass module (SPMD; all cores run the same IR)."""
    import sys

    if "/opt/trn_rl_repo" not in sys.path:
        sys.path.insert(0, "/opt/trn_rl_repo")

    import concourse.tile as tile
    from concourse import bacc, mybir
    from concourse.masks import make_identity
    from contextlib import ExitStack

    f32 = mybir.dt.float32
    bf16 = mybir.dt.bfloat16
    i16 = mybir.dt.int16
    B = B_TOT

    nc = bacc.Bacc("TRN2", target_bir_lowering=False)
    xT = nc.declare_dram_parameter("xTb", [IN_DIM, B], bf16, isOutput=False)
    wgt = nc.declare_dram_parameter("wgt_shuf", [128, RPT * 16], f32, isOutput=False)
    cg = nc.declare_dram_parameter("cgate", [128, 64], f32, isOutput=False)
    idxab = nc.declare_dram_parameter("idxab16", [128, NG * 32], i16, isOutput=False)
    out = nc.declare_dram_parameter("out", [B, OC], bf16, isOutput=True)

    Ident = mybir.ActivationFunctionType.Identity
    Exp = mybir.ActivationFunctionType.Exp
    MULT = mybir.AluOpType.mult
    ADD = mybir.AluOpType.add

    with tile.TileContext(nc) as tc, ExitStack() as ctx:
        cpool = ctx.enter_context(tc.tile_pool(name="consts", bufs=1))
        idx_pool = ctx.enter_context(tc.tile_pool(name="idxp", bufs=1))
        wpool = ctx.enter_context(tc.tile_pool(name="wtmp", bufs=2))

        # idx first: the gathers are the head of the critical path
        idx_sb = idx_pool.tile([128, NG * 32], i16, name="idx_sb")
        nc.sync.dma_start(idx_sb[:], idxab[:])
        cgt = cpool.tile([128, 64], f32, name="cgt")
        nc.sync.dma_start(cgt[:], cg[:])
        wtile = wpool.tile([128, RPT * 16], f32, name="wtile")
        nc.sync.dma_start(wtile[:], wgt[:])

        # ---- gathers: one per 256-neuron group, a+b combined ----------
        gpool = ctx.enter_context(tc.tile_pool(name="gath", bufs=4))
        gt = []
        for gk in range(NG):
            g = gpool.tile([128, 4, B], bf16, name=f"g{gk}", tag="g")
            nc.gpsimd.dma_gather(
                g[:], xT[:], idx_sb[:, gk * 32:(gk + 1) * 32], 2 * GJ, 2 * GJ, B
            )
            gt.append(g)

        identb = cpool.tile([128, 128], bf16, name="identb")
        make_identity(nc, identb[:])

        # ---- W = softmax(weights) @ C, in (q, r) layout: j = r*128 + q ----
        wk = [cpool.tile([128, RPT], f32, name=f"wk{k}") for k in range(4)]
        wexp = wpool.tile([128, RPT * 16], f32, name="wexp")
        nc.scalar.activation(wexp[:], wtile[:], Exp)
        wsum = wpool.tile([128, RPT], f32, name="wsum")
        nc.vector.tensor_reduce(
            out=wsum[:],
            in_=wexp[:].rearrange("p (r k) -> p r k", k=16),
            op=ADD,
            axis=mybir.AxisListType.X,
        )
        wrcp = wpool.tile([128, RPT], f32, name="wrcp")
        nc.vector.reciprocal(wrcp[:], wsum[:])
        for k in range(4):
            wtmp = wpool.tile([128, RPT * 16], f32, name="wtmp", tag="wtmp")
            ck_bcast = (
                cgt[:, k * 16:(k + 1) * 16]
                .rearrange("p (r k) -> p r k", r=1)
                .to_broadcast([128, RPT, 16])
            )
            nc.vector.tensor_tensor(
                out=wtmp[:].rearrange("p (r k) -> p r k", k=16),
                in0=wexp[:].rearrange("p (r k) -> p r k", k=16),
                in1=ck_bcast,
                op=MULT,
            )
            wred = wpool.tile([128, RPT], f32, name="wred", tag="wred")
            nc.vector.tensor_reduce(
                out=wred[:],
                in_=wtmp[:].rearrange("p (r k) -> p r k", k=16),
                op=ADD,
                axis=mybir.AxisListType.X,
            )
            nc.vector.tensor_tensor(out=wk[k][:], in0=wred[:], in1=wrcp[:],
                                    op=MULT)

        # ---- per-slot gates + transpose-back --------------------------
        uvpool = ctx.enter_context(tc.tile_pool(name="uv", bufs=6))
        prepool = ctx.enter_context(tc.tile_pool(name="pre", bufs=10))
        psumO = ctx.enter_context(tc.tile_pool(name="psumO", bufs=4, space="PSUM"))
        ostg = ctx.enter_context(tc.tile_pool(name="ostg", bufs=4))

        pre = [None] * RPT
        for gk in range(NG):
            g = gt[gk]
            for cj in range(2):
                r = gk * 2 + cj
                u = uvpool.tile([128, B], bf16, tag="u")
                nc.scalar.activation(
                    u[:], g[:, cj, :], Ident,
                    scale=wk[3][:, r:r + 1], bias=wk[2][:, r:r + 1],
                )
                v = uvpool.tile([128, B], bf16, tag="v")
                nc.vector.tensor_scalar(
                    v[:], g[:, cj, :],
                    wk[1][:, r:r + 1], wk[0][:, r:r + 1],
                    op0=MULT, op1=ADD,
                )
                t = uvpool.tile([128, B], bf16, tag="t")
                nc.vector.tensor_tensor(t[:], u[:], g[:, 2 + cj, :], op=MULT)
                p = prepool.tile([128, B], bf16, tag="p")
                nc.vector.tensor_tensor(p[:], t[:], v[:], op=ADD)
                pre[r] = p
            if gk % 2 == 1:
                half = gk // 2
                for h in range(NH):
                    po = psumO.tile([128, 512], bf16, tag="po")
                    for s in range(4):
                        nc.tensor.transpose(
                            po[:, s * 128:(s + 1) * 128],
                            pre[half * 4 + s][:, h * 128:(h + 1) * 128],
                            identb[:],
                        )
                    og = ostg.tile([128, 512], bf16, tag="og")
                    if h % 2 == 0:
                        nc.scalar.copy(og[:], po[:])
                    else:
                        nc.vector.tensor_copy(og[:], po[:])
                    nc.sync.dma_start(
                        out[h * 128:(h + 1) * 128,
                            half * 512:(half + 1) * 512], og[:]
                    )
    nc.compile()
    return nc


# ---------------------------------------------------------------- host side
def _wrap_idx_core(ia_core, ib_core):
    """Pack one core's combined a/b indices into the wrapped int16 layout.

    Gather gk covers neurons [gk*GJ, (gk+1)*GJ) of this core's shard and
    pulls 2*GJ rows: the GJ idx_a rows then the GJ idx_b rows. Unwrapped
    position i = s*16 + p reads idx16[p % 16, gk*32 + s]; replicated over
    the 8 groups of 16 partitions.
    """
    comb = np.stack(
        [np.concatenate([ia_core[gk * GJ:(gk + 1) * GJ],
                         ib_core[gk * GJ:(gk + 1) * GJ]])
         for gk in range(NG)]
    ).astype(np.int16)  # [NG, 2*GJ]
    w = comb.reshape(NG, 32, 16).transpose(2, 0, 1).reshape(16, NG * 32)
    return np.ascontiguousarray(np.tile(w, (8, 1)))  # [128, NG*32]


def _prep_inputs(x, weights, idx_a, idx_b):
    import ml_dtypes

    x = np.asarray(x, dtype=np.float32)
    weights = np.asarray(weights, dtype=np.float32)
    idx_a = np.asarray(idx_a)
    idx_b = np.asarray(idx_b)
    xTb = np.ascontiguousarray(x.T.astype(ml_dtypes.bfloat16))  # [IN, B] bf16
    cgate = np.ascontiguousarray(np.tile(GATE_C.T.reshape(1, 64), (128, 1)))
    in_maps = []
    for c in range(NCORES):
        wc = weights[c * OC:(c + 1) * OC]              # [OC, 16]
        wgt_shuf = np.ascontiguousarray(
            wc.reshape(RPT, 128, 16).transpose(1, 0, 2).reshape(128, -1)
        )
        iw = _wrap_idx_core(idx_a[c * OC:(c + 1) * OC],
                            idx_b[c * OC:(c + 1) * OC])
        in_maps.append(
            {
                "xTb": xTb,
                "wgt_shuf": wgt_shuf,
                "cgate": cgate,
                "idxab16": iw,
            }
        )
    return in_maps


_NC_CACHE = {}


def _get_nc():
    if "nc" not in _NC_CACHE:
        _NC_CACHE["nc"] = build_nc()
    return _NC_CACHE["nc"]


def kernel(x, weights, idx_a, idx_b):
    import sys

    if "/opt/trn_rl_repo" not in sys.path:
        sys.path.insert(0, "/opt/trn_rl_repo")
    from concourse.bass_utils import run_bass_kernel_spmd

    nc = _get_nc()
    in_maps = _prep_inputs(x, weights, idx_a, idx_b)
    res = run_bass_kernel_spmd(nc, in_maps, list(range(NCORES)))
    return np.concatenate(
        [r["out"].astype(np.float32) for r in res.results], axis=1
    )


if __name__ == "__main__":
    nc = build_nc()
    print("built OK")


# revision 14
# speedup vs baseline: 5.9698x; 1.0574x over previous
"""Trainium2 Bass kernel for nn_LogicLayer (differentiable logic-gate layer).

Reference computation:
    a = x[:, idx_a]; b = x[:, idx_b]                  # [B, OUT] gathers
    w = softmax(weights, -1)                          # [OUT, 16]
    out = sum_k w[:, k] * gate_k(a, b)

Every gate value is of the form c0 + c1*a + c2*b + c3*a*b, so
    out[i, j] = W0[j] + W1[j]*a + W2[j]*b + W3[j]*a*b
with W = softmax(weights) @ C, C the [16, 4] gate-coefficient table.

Sharding: out_dim-parallel across 8 cores (1024 neurons each, full 2048
batch), x feature-major (transposed) in bf16 and replicated. This keeps
the SWDGE gather descriptor count per core at 2048 (the Q7 descriptor
loop costs ~8.5 ns/index, so batch-parallel's 16K indices/core was the
bottleneck) while HBM traffic stays 8 MiB gathers + 8 MiB stores/core.

Device pipeline per core:
  - W = softmax(weights) @ C on ACT/DVE (overlaps the first gathers)
  - 4 SWDGE dma_gathers, each pulling 256 neurons' a- and b-columns
    (512 indices x 4 KiB bf16 rows, neurons land on partitions)
  - per 128-neuron slot: u = W3*a + W2 (ACT), v = W1*a + W0 (DVE),
    t = u*b (DVE), pre = t + v (DVE), all bf16
  - PE transposes pre back to batch-major (identity matmuls, bf16 PSUM),
    ACT/DVE copy-cast to f32, HWDGE stores [128, 512] blocks.
"""

import numpy as np

# ---------------------------------------------------------------- constants
B_TOT, IN_DIM, OUT_DIM = 2048, 8192, 8192
NCORES = 8
OC = OUT_DIM // NCORES       # 1024 neurons per core
NG = 4                       # gather groups per core (256 neurons each)
GJ = OC // NG                # 256 neurons per gather
RPT = OC // 128              # 8 (W free dim per partition)
NH = B_TOT // 128            # 16 batch blocks

# value = c0 + c1*a + c2*b + c3*ab  for each of the 16 gates
GATE_C = np.array(
    [
        # c0  c1  c2  c3
        [0, 0, 0, 0],    # 0  False
        [0, 0, 0, 1],    # 1  a AND b
        [0, 1, 0, -1],   # 2  a AND NOT b
        [0, 1, 0, 0],    # 3  a
        [0, 0, 1, -1],   # 4  NOT a AND b
        [0, 0, 1, 0],    # 5  b
        [0, 1, 1, -2],   # 6  a XOR b
        [0, 1, 1, -1],   # 7  a OR b
        [1, -1, -1, 1],  # 8  NOT (a OR b)
        [1, -1, -1, 2],  # 9  NOT (a XOR b)
        [1, 0, -1, 0],   # 10 NOT b
        [1, 0, -1, 1],   # 11 a OR NOT b
        [1, -1, 0, 0],   # 12 NOT a
        [1, -1, 0, 1],   # 13 NOT a OR b
        [1, 0, 0, -1],   # 14 NOT (a AND b)
        [1, 0, 0, 0],    # 15 True
    ],
    dtype=np.float32,
)  # [16, 4]


# ---------------------------------------------------------------- device IR
def build_nc():
    """Build the per-core Bass module (SPMD; all cores run the same IR)."""
    import sys

    if "/opt/trn_rl_repo" not in sys.path:
        sys.path.insert(0, "/opt/trn_rl_repo")

    import concourse.tile as tile
    from concourse import bacc, mybir
    from concourse.masks import make_identity
    from contextlib import ExitStack

    f32 = mybir.dt.float32
    bf16 = mybir.dt.bfloat16
    i16 = mybir.dt.int16
    B = B_TOT

    nc = bacc.Bacc("TRN2", target_bir_lowering=False)
    xT = nc.declare_dram_parameter("xTb", [IN_DIM, B], bf16, isOutput=False)
    # one packed small-input param: cgate f32 [64] | wgt_shuf f32 [128] |
    # idxab16 i16 [128] as 64 f32 cols — a single DMA so the first gather's
    # dependency chain is one transfer, not three serialized ones
    pk = nc.declare_dram_parameter("pk", [128, 64 + RPT * 16 + NG * 16], f32,
                                   isOutput=False)
    out = nc.declare_dram_parameter("out", [B, OC], bf16, isOutput=True)

    Ident = mybir.ActivationFunctionType.Identity
    Exp = mybir.ActivationFunctionType.Exp
    MULT = mybir.AluOpType.mult
    ADD = mybir.AluOpType.add

    with tile.TileContext(nc) as tc, ExitStack() as ctx:
        cpool = ctx.enter_context(tc.tile_pool(name="consts", bufs=1))
        idx_pool = ctx.enter_context(tc.tile_pool(name="idxp", bufs=1))
        wpool = ctx.enter_context(tc.tile_pool(name="wtmp", bufs=2))

        pkt = idx_pool.tile([128, 64 + RPT * 16 + NG * 16], f32, name="pkt")
        nc.sync.dma_start(pkt[:], pk[:])
        cgt = pkt[:, 0:64]
        wtile = pkt[:, 64:64 + RPT * 16]
        idx_sb = pkt[:, 64 + RPT * 16:].bitcast(i16)  # [128, NG*32] i16

        # ---- gathers: one per 256-neuron group, a+b combined ----------
        gpool = ctx.enter_context(tc.tile_pool(name="gath", bufs=4))
        gt = []
        for gk in range(NG):
            g = gpool.tile([128, 4, B], bf16, name=f"g{gk}", tag="g")
            nc.gpsimd.dma_gather(
                g[:], xT[:], idx_sb[:, gk * 32:(gk + 1) * 32], 2 * GJ, 2 * GJ, B
            )
            gt.append(g)

        identb = cpool.tile([128, 128], bf16, name="identb")
        make_identity(nc, identb[:])

        # ---- W = softmax(weights) @ C, in (q, r) layout: j = r*128 + q ----
        wk = [cpool.tile([128, RPT], f32, name=f"wk{k}") for k in range(4)]
        wexp = wpool.tile([128, RPT * 16], f32, name="wexp")
        nc.scalar.activation(wexp[:], wtile, Exp)
        wsum = wpool.tile([128, RPT], f32, name="wsum")
        nc.vector.tensor_reduce(
            out=wsum[:],
            in_=wexp[:].rearrange("p (r k) -> p r k", k=16),
            op=ADD,
            axis=mybir.AxisListType.X,
        )
        wrcp = wpool.tile([128, RPT], f32, name="wrcp")
        nc.vector.reciprocal(wrcp[:], wsum[:])
        for k in range(4):
            wtmp = wpool.tile([128, RPT * 16], f32, name="wtmp", tag="wtmp")
            ck_bcast = (
                cgt[:, k * 16:(k + 1) * 16]
                .rearrange("p (r k) -> p r k", r=1)
                .to_broadcast([128, RPT, 16])
            )
            nc.vector.tensor_tensor(
                out=wtmp[:].rearrange("p (r k) -> p r k", k=16),
                in0=wexp[:].rearrange("p (r k) -> p r k", k=16),
                in1=ck_bcast,
                op=MULT,
            )
            wred = wpool.tile([128, RPT], f32, name="wred", tag="wred")
            nc.vector.tensor_reduce(
                out=wred[:],
                in_=wtmp[:].rearrange("p (r k) -> p r k", k=16),
                op=ADD,
                axis=mybir.AxisListType.X,
            )
            nc.vector.tensor_tensor(out=wk[k][:], in0=wred[:], in1=wrcp[:],
                                    op=MULT)

        # ---- per-slot gates + transpose-back --------------------------
        uvpool = ctx.enter_context(tc.tile_pool(name="uv", bufs=6))
        prepool = ctx.enter_context(tc.tile_pool(name="pre", bufs=10))
        psumO = ctx.enter_context(tc.tile_pool(name="psumO", bufs=6, space="PSUM"))
        ostg = ctx.enter_context(tc.tile_pool(name="ostg", bufs=8))

        pre = [None] * RPT
        for gk in range(NG):
            g = gt[gk]
            for cj in range(2):
                r = gk * 2 + cj
                u = uvpool.tile([128, B], bf16, tag="u")
                nc.scalar.activation(
                    u[:], g[:, cj, :], Ident,
                    scale=wk[3][:, r:r + 1], bias=wk[2][:, r:r + 1],
                )
                v = uvpool.tile([128, B], bf16, tag="v")
                nc.vector.tensor_scalar(
                    v[:], g[:, cj, :],
                    wk[1][:, r:r + 1], wk[0][:, r:r + 1],
                    op0=MULT, op1=ADD,
                )
                t = uvpool.tile([128, B], bf16, tag="t")
                nc.vector.tensor_tensor(t[:], u[:], g[:, 2 + cj, :], op=MULT)
                p = prepool.tile([128, B], bf16, tag="p")
                nc.vector.tensor_tensor(p[:], t[:], v[:], op=ADD)
                pre[r] = p
            if gk % 2 == 1:
                half = gk // 2
                for h in range(NH):
                    po = psumO.tile([128, 512], bf16, tag="po")
                    for s in range(4):
                        nc.tensor.transpose(
                            po[:, s * 128:(s + 1) * 128],
                            pre[half * 4 + s][:, h * 128:(h + 1) * 128],
                            identb[:],
                        )
                    og = ostg.tile([128, 512], bf16, tag="og")
                    if h % 2 == 0:
                        nc.scalar.copy(og[:], po[:])
                    else:
                        nc.vector.tensor_copy(og[:], po[:])
                    nc.sync.dma_start(
                        out[h * 128:(h + 1) * 128,
                            half * 512:(half + 1) * 512], og[:]
                    )
    nc.compile()
    return nc


# ---------------------------------------------------------------- host side
def _wrap_idx_core(ia_core, ib_core):
    """Pack one core's combined a/b indices into the wrapped int16 layout.

    Gather gk covers neurons [gk*GJ, (gk+1)*GJ) of this core's shard and
    pulls 2*GJ rows: the GJ idx_a rows then the GJ idx_b rows. Unwrapped
    position i = s*16 + p reads idx16[p % 16, gk*32 + s]; replicated over
    the 8 groups of 16 partitions.
    """
    comb = np.stack(
        [np.concatenate([ia_core[gk * GJ:(gk + 1) * GJ],
                         ib_core[gk * GJ:(gk + 1) * GJ]])
         for gk in range(NG)]
    ).astype(np.int16)  # [NG, 2*GJ]
    w = comb.reshape(NG, 32, 16).transpose(2, 0, 1).reshape(16, NG * 32)
    return np.ascontiguousarray(np.tile(w, (8, 1)))  # [128, NG*32]


def _prep_inputs(x, weights, idx_a, idx_b):
    import ml_dtypes

    x = np.asarray(x, dtype=np.float32)
    weights = np.asarray(weights, dtype=np.float32)
    idx_a = np.asarray(idx_a)
    idx_b = np.asarray(idx_b)
    xTb = np.ascontiguousarray(x.T.astype(ml_dtypes.bfloat16))  # [IN, B] bf16
    cgate = np.tile(GATE_C.T.reshape(1, 64), (128, 1))
    in_maps = []
    for c in range(NCORES):
        wc = weights[c * OC:(c + 1) * OC]              # [OC, 16]
        wgt_shuf = wc.reshape(RPT, 128, 16).transpose(1, 0, 2).reshape(128, -1)
        iw = _wrap_idx_core(idx_a[c * OC:(c + 1) * OC],
                            idx_b[c * OC:(c + 1) * OC])
        pk = np.ascontiguousarray(
            np.concatenate([cgate, wgt_shuf, iw.view(np.float32)], axis=1)
        )
        in_maps.append({"xTb": xTb, "pk": pk})
    return in_maps


_NC_CACHE = {}


def _get_nc():
    if "nc" not in _NC_CACHE:
        _NC_CACHE["nc"] = build_nc()
    return _NC_CACHE["nc"]


def kernel(x, weights, idx_a, idx_b):
    import sys

    if "/opt/trn_rl_repo" not in sys.path:
        sys.path.insert(0, "/opt/trn_rl_repo")
    from concourse.bass_utils import run_bass_kernel_spmd

    nc = _get_nc()
    in_maps = _prep_inputs(x, weights, idx_a, idx_b)
    res = run_bass_kernel_spmd(nc, in_maps, list(range(NCORES)))
    return np.concatenate(
        [r["out"].astype(np.float32) for r in res.results], axis=1
    )


if __name__ == "__main__":
    nc = build_nc()
    print("built OK")
